# revision 1
# baseline (speedup 1.0000x reference)
"""Causal multi-head attention on 8 TRN2 NeuronCores.

Problem: B=4, S=2048, D=1024, H=16, HD=64, causal MHA with out-proj + bias.

Sharding: core c handles (batch b = c//2, head-half hh = c%2), i.e. 8 heads of
one batch element. Per core:
  Q^T/K^T = (Wq/Wk half)^T X_b^T   -> [64*2, S] per head pair (head on partition)
  V       = X_b @ Wv half          -> [S, 8*65] (65 = 64 + fused-ones column)
  S^T_j   = K_j Q^T (scores transposed: keys on partition) per 128-key block
  P^T     = exp(S^T/8) (ScalarE), causal handled by block skipping + one
            [128,128] additive mask on diagonal blocks
  ctx'^T  = V'^T P^T accumulated over key blocks; row 64 = softmax denominators
            (ones-column trick)
  ctxT    = ctx'^T * (1/denom) broadcast  (GPSIMD partition_broadcast + DVE)
  out     = ctxT^T @ Wo half  (partial; host sums the two half partials + bias)

All matmuls in bf16 (fp32 accumulate in PSUM); scores/softmax in fp32.
"""

import numpy as np
import ml_dtypes
from contextlib import ExitStack

import concourse.bass as bass
import concourse.bacc as bacc
import concourse.mybir as mybir
import concourse.tile as tile
from concourse import bass_utils

F32 = mybir.dt.float32
BF16 = mybir.dt.bfloat16

B, S, D = 4, 2048, 1024
H, HD = 16, 64
DH = 512          # columns of the head-half handled by one core (8 heads * 64)
NCORES = 8
CH = 512          # q chunk width
NCH = S // CH     # 4
NKB = S // 128    # 16 key/query 128-blocks
KT = D // 128     # 8 contraction tiles for the projections
NEG = -30000.0    # additive mask value (exp(0.125*NEG) == 0 in fp32)

_CACHED = None


def build_module():
    nc = bacc.Bacc("TRN2", target_bir_lowering=False, debug=False)

    xT = nc.dram_tensor("xT", [D, S], BF16, kind="ExternalInput")
    wq = nc.dram_tensor("wq", [D, DH], BF16, kind="ExternalInput")
    wk = nc.dram_tensor("wk", [D, DH], BF16, kind="ExternalInput")
    wv = nc.dram_tensor("wv", [D, DH], BF16, kind="ExternalInput")
    wo = nc.dram_tensor("wo", [DH, D], BF16, kind="ExternalInput")
    maskt = nc.dram_tensor("maskt", [128, 128], F32, kind="ExternalInput")
    out = nc.dram_tensor("out", [S, D], F32, kind="ExternalOutput")
    # DRAM bounce rows for the reciprocal partition-broadcast (DMA cannot
    # broadcast an SBUF source, but a DRAM source row replicates fine)
    rscratch = nc.dram_tensor("rscratch", [4 * NCH * 2, CH], F32, kind="Internal")

    with tile.TileContext(nc) as tc, ExitStack() as ctx:
        const = ctx.enter_context(tc.tile_pool(name="const", bufs=1))
        xTp = ctx.enter_context(tc.tile_pool(name="xTp", bufs=1))
        wp = ctx.enter_context(tc.tile_pool(name="wp", bufs=1))
        qkp = ctx.enter_context(tc.tile_pool(name="qkp", bufs=1))
        vp = ctx.enter_context(tc.tile_pool(name="vp", bufs=1))
        ctp = ctx.enter_context(tc.tile_pool(name="ctp", bufs=1))
        pTp = ctx.enter_context(tc.tile_pool(name="pTp", bufs=6))
        rp = ctx.enter_context(tc.tile_pool(name="rp", bufs=3))
        bp = ctx.enter_context(tc.tile_pool(name="bp", bufs=3))
        osb = ctx.enter_context(tc.tile_pool(name="osb", bufs=3))
        ps_sc = ctx.enter_context(tc.tile_pool(name="ps_sc", bufs=4, space="PSUM"))
        ps_ctx = ctx.enter_context(tc.tile_pool(name="ps_ctx", bufs=2, space="PSUM"))
        ps_mm = ctx.enter_context(tc.tile_pool(name="ps_mm", bufs=2, space="PSUM"))

        mask = const.tile([128, 128], F32, name="mask", tag="mask")
        nc.sync.dma_start(mask[:], maskt[:])

        # --- input loads -------------------------------------------------
        def load_w(name, dram, cols):
            ts = []
            for k in range(dram.shape[0] // 128):
                t = wp.tile([128, cols], BF16, name=f"{name}{k}", tag=f"{name}{k}")
                nc.sync.dma_start(t[:], dram[128 * k:128 * (k + 1), :])
                ts.append(t)
            return ts

        wk_t = load_w("wk", wk, DH)
        wq_t = load_w("wq", wq, DH)
        wv_t = load_w("wv", wv, DH)

        xts = []
        for k in range(KT):
            t = xTp.tile([128, S], BF16, name=f"xT{k}", tag=f"xT{k}")
            xts.append(t)
        # chunk-outer loads: all k-tiles of chunk 0 land first so the first
        # projection matmuls can start ~12us earlier
        for c in range(NCH):
            for k in range(KT):
                nc.sync.dma_start(xts[k][:, CH * c:CH * (c + 1)],
                                  xT[128 * k:128 * (k + 1), CH * c:CH * (c + 1)])

        wo_t = load_w("wo", wo, D)

        # --- persistent intermediates ------------------------------------
        # Q^T / K^T per head pair g: [128 (= 2 heads x 64), S]
        qts = [qkp.tile([128, S], BF16, name=f"qt{g}", tag=f"qt{g}") for g in range(4)]
        kts = [qkp.tile([128, S], BF16, name=f"kt{g}", tag=f"kt{g}") for g in range(4)]
        # V with interleaved ones column: [128 keys, 8 heads * 65]
        vts = [vp.tile([128, 8 * 65], BF16, name=f"v{m}", tag=f"v{m}") for m in range(NKB)]
        # normalized ctx^T per pair g: rows 0:64 head 2g, 64:128 head 2g+1
        cts = [ctp.tile([128, S], BF16, name=f"ct{g}", tag=f"ct{g}") for g in range(4)]

        def proj_qk_chunk(dst, w_tiles, g, c):
            """dst[:, CH*c:CH*(c+1)] = (W pair cols)^T X^T chunk, bf16."""
            ps = ps_mm.tile([128, CH], F32, name="mm", tag="mm")
            for k in range(KT):
                nc.tensor.matmul(
                    ps[:],
                    lhsT=w_tiles[k][:, 128 * g:128 * (g + 1)],
                    rhs=xts[k][:, CH * c:CH * (c + 1)],
                    start=(k == 0), stop=(k == KT - 1),
                )
            nc.vector.tensor_copy(dst[:, CH * c:CH * (c + 1)], ps[:])

        def proj_v_block(m):
            """V rows [128m, 128m+128) for all 8 heads, strided into vts[m]."""
            ps = ps_mm.tile([128, CH], F32, name="mm", tag="mm")
            for k in range(KT):
                nc.tensor.matmul(
                    ps[:],
                    lhsT=xts[k][:, 128 * m:128 * (m + 1)],
                    rhs=wv_t[k][:],
                    start=(k == 0), stop=(k == KT - 1),
                )
            vm = vts[m]
            dst = vm[:].rearrange("p (h x) -> p h x", x=65)[:, :, 0:64]
            src = ps[:].rearrange("p (h d) -> p h d", d=64)
            nc.vector.tensor_copy(dst, src)
            ones = vm[:].rearrange("p (h x) -> p h x", x=65)[:, :, 64:65]
            nc.vector.memset(ones, 1.0)

        def attention_chunk(g, c):
            """Heads (2g, 2g+1), queries [CH*c, CH*(c+1))."""
            qt, ktile = qts[g], kts[g]
            nj = 4 * c + 4
            ctx_e = ps_ctx.tile([65, CH], F32, name="ctx", tag="ctx")
            ctx_o = ps_ctx.tile([65, CH], F32, name="ctx", tag="ctx")
            for j in range(nj):
                d = j - 4 * c
                st = 128 * max(0, d)  # first valid q column in this chunk
                sc_e = ps_sc.tile([128, CH], F32, name="sc", tag="sc")
                sc_o = ps_sc.tile([128, CH], F32, name="sc", tag="sc")
                # scores^T = K_j Q^T  (row-packed pair: even rows 0:64, odd 64:128)
                nc.tensor.matmul(
                    sc_e[:, st:], lhsT=ktile[0:64, 128 * j:128 * (j + 1)],
                    rhs=qt[0:64, CH * c + st:CH * (c + 1)])
                nc.tensor.matmul(
                    sc_o[:, st:], lhsT=ktile[64:128, 128 * j:128 * (j + 1)],
                    rhs=qt[64:128, CH * c + st:CH * (c + 1)])
                if d >= 0:  # diagonal-crossing block: triangular mask
                    ms = slice(128 * d, 128 * (d + 1))
                    nc.vector.tensor_tensor(sc_e[:, ms], sc_e[:, ms], mask[:],
                                            op=mybir.AluOpType.add)
                    nc.vector.tensor_tensor(sc_o[:, ms], sc_o[:, ms], mask[:],
                                            op=mybir.AluOpType.add)
                pt_e = pTp.tile([128, CH], BF16, name="pT", tag="pT")
                pt_o = pTp.tile([128, CH], BF16, name="pT", tag="pT")
                nc.scalar.activation(pt_e[:, st:], sc_e[:, st:],
                                     mybir.ActivationFunctionType.Exp, scale=0.125)
                nc.scalar.activation(pt_o[:, st:], sc_o[:, st:],
                                     mybir.ActivationFunctionType.Exp, scale=0.125)
                he, ho = 2 * g, 2 * g + 1
                nc.tensor.matmul(ctx_e[:, st:], lhsT=vts[j][:, 65 * he:65 * he + 65],
                                 rhs=pt_e[:, st:], start=(j == 0), stop=(j == nj - 1),
                                 skip_group_check=True)
                nc.tensor.matmul(ctx_o[:, st:], lhsT=vts[j][:, 65 * ho:65 * ho + 65],
                                 rhs=pt_o[:, st:], start=(j == 0), stop=(j == nj - 1),
                                 skip_group_check=True)
            # normalize by the fused denominator row and store into ctxT (bf16).
            # Evict ctx' to SBUF in one copy first: frees the PSUM bank after
            # ~0.7us instead of holding it through the DMA-bounce round trip
            # (which stalled the next chunk's PV and re-throttled the PE), and
            # gives the custom-DVE reciprocal an SBUF source (PSUM source
            # reads garbage on HW).
            for par, (ctx_ps, rows) in enumerate(
                    ((ctx_e, slice(0, 64)), (ctx_o, slice(64, 128)))):
                idx = (g * NCH + c) * 2 + par
                stg = rp.tile([65, CH], F32, name="stg", tag="stg")
                nc.vector.tensor_copy(stg[:], ctx_ps[:])
                # custom-DVE input must sit at partition 0: copy the sums row
                srow = rp.tile([1, CH], F32, name="srow", tag="srow")
                nc.vector.tensor_copy(srow[:], stg[64:65, :])
                rc = rp.tile([1, CH], F32, name="recip", tag="recip")
                nc.vector.reciprocal_approx_fast(rc[:], srow[:])
                nc.sync.dma_start(rscratch[idx:idx + 1, :], rc[:])
                bc = bp.tile([64, CH], F32, name="bcast", tag="bcast")
                nc.sync.dma_start(bc[:], rscratch[idx:idx + 1, :].broadcast_to([64, CH]))
                nc.vector.tensor_tensor(cts[g][rows, CH * c:CH * (c + 1)],
                                        stg[0:64, :], bc[:],
                                        op=mybir.AluOpType.mult)

        def outproj_chunk(c):
            for qb in range(4 * c, 4 * c + 4):
                for n in range(2):
                    ps = ps_mm.tile([128, CH], F32, name="mm", tag="mm")
                    for g in range(4):
                        nc.tensor.matmul(
                            ps[:], lhsT=cts[g][:, 128 * qb:128 * (qb + 1)],
                            rhs=wo_t[g][:, CH * n:CH * (n + 1)],
                            start=(g == 0), stop=(g == 3),
                        )
                    ot = osb.tile([128, CH], F32, name="osb", tag="osb")
                    nc.vector.tensor_copy(ot[:], ps[:])
                    nc.sync.dma_start(out[128 * qb:128 * (qb + 1), CH * n:CH * (n + 1)],
                                      ot[:])

        # --- schedule ----------------------------------------------------
        for m in range(NKB):
            proj_v_block(m)
        for c in range(NCH):
            proj_qk_chunk(kts[0], wk_t, 0, c)
            proj_qk_chunk(qts[0], wq_t, 0, c)
        for g in range(4):
            for c in range(NCH):
                attention_chunk(g, c)
                if g < 3:  # produce next pair's Q^T/K^T while ACT chews exps
                    proj_qk_chunk(kts[g + 1], wk_t, g + 1, c)
                    proj_qk_chunk(qts[g + 1], wq_t, g + 1, c)
                else:
                    outproj_chunk(c)

    nc.compile()
    return nc


def _get_module():
    global _CACHED
    if _CACHED is None:
        _CACHED = build_module()
    return _CACHED


def _causal_mask_tile():
    k = np.arange(128)[:, None]
    q = np.arange(128)[None, :]
    return np.where(k <= q, 0.0, NEG).astype(np.float32)


def kernel(inputs, Wq, Wk, Wv, Wo, bo):
    inputs = np.asarray(inputs, dtype=np.float32)
    Wq = np.asarray(Wq, dtype=np.float32)
    Wk = np.asarray(Wk, dtype=np.float32)
    Wv = np.asarray(Wv, dtype=np.float32)
    Wo = np.asarray(Wo, dtype=np.float32)
    bo = np.asarray(bo, dtype=np.float32)

    bf = ml_dtypes.bfloat16
    mask = _causal_mask_tile()
    in_maps = []
    for c in range(NCORES):
        b, hh = c // 2, c % 2
        cols = slice(DH * hh, DH * (hh + 1))
        in_maps.append({
            "xT": np.ascontiguousarray(inputs[b].T).astype(bf),
            "wq": np.ascontiguousarray(Wq[:, cols]).astype(bf),
            "wk": np.ascontiguousarray(Wk[:, cols]).astype(bf),
            "wv": np.ascontiguousarray(Wv[:, cols]).astype(bf),
            "wo": np.ascontiguousarray(Wo[cols, :]).astype(bf),
            "maskt": mask,
        })

    nc = _get_module()
    res = bass_utils.run_bass_kernel_spmd(nc, in_maps, core_ids=list(range(NCORES)))
    outs = [r["out"] for r in res.results]

    full = np.empty((B, S, D), dtype=np.float32)
    for b in range(B):
        full[b] = outs[2 * b] + outs[2 * b + 1] + bo[None, :]
    return full



# revision 2
# speedup vs baseline: 1.0615x; 1.0615x over previous
"""Causal multi-head attention on 8 TRN2 NeuronCores.

Problem: B=4, S=2048, D=1024, H=16, HD=64, causal MHA with out-proj + bias.

Sharding: core c handles (batch b = c//2, head-half hh = c%2), i.e. 8 heads of
one batch element. Per core:
  Q^T/K^T = (Wq/Wk half)^T X_b^T   -> [64*2, S] per head pair (head on partition)
  V       = X_b @ Wv half          -> [S, 8*65] (65 = 64 + fused-ones column)
  S^T_j   = K_j Q^T (scores transposed: keys on partition) per 128-key block
  P^T     = exp(S^T/8) (ScalarE), causal handled by block skipping + one
            [128,128] additive mask on diagonal blocks
  ctx'^T  = V'^T P^T accumulated over key blocks; row 64 = softmax denominators
            (ones-column trick)
  ctxT    = ctx'^T * (1/denom) broadcast  (DRAM-bounce partition broadcast)
  out     = ctxT^T @ Wo half  (partial; host sums the two half partials + bias)

Scheduling (the perf-critical part):
  - Inputs arrive in ~11 large host-packed DMAs ordered so the first
    projection can start ~8us in (each dma_start costs ~600ns serial on SP).
  - The attention inner loop is software-pipelined: scores(j+1) is issued
    before PV(j), so the PE never waits for the ACT-engine exp. Projection /
    out-proj matmuls are interleaved as filler between steps to keep the PE
    gapless (which also holds it at the 2.4GHz p-state).
  - Even/odd-head score tiles share one 2-bank PSUM tile [128,1024] so a
    single ACT instruction exps both, halving ACT per-instruction bubbles.

All matmuls in bf16 (fp32 accumulate in PSUM); scores/softmax in fp32.
"""

import numpy as np
import ml_dtypes
from contextlib import ExitStack

import concourse.bass as bass
import concourse.bacc as bacc
import concourse.mybir as mybir
import concourse.tile as tile
from concourse import bass_utils

F32 = mybir.dt.float32
BF16 = mybir.dt.bfloat16

B, S, D = 4, 2048, 1024
H, HD = 16, 64
DH = 512          # columns of the head-half handled by one core (8 heads * 64)
NCORES = 8
CH = 512          # q chunk width
NCH = S // CH     # 4
NKB = S // 128    # 16 key/query 128-blocks
KT = D // 128     # 8 contraction tiles for the projections
NEG = -30000.0    # additive mask value (exp(0.125*NEG) == 0 in fp32)

_CACHED = None


def build_module():
    nc = bacc.Bacc("TRN2", target_bir_lowering=False, debug=False)

    # --- dram tensors: host-packed for few, large, contiguous DMAs --------
    # x chunk-major: xc{c}[p, 512*k + s] = X^T[128k+p, 512c+s]
    xcd = [nc.dram_tensor(f"xc{c}", [128, KT * CH], BF16, kind="ExternalInput")
           for c in range(NCH)]
    # wk/wq pair-major, split so pair 0 can arrive first:
    #   wk0[p, 128k+i] = Wk[128k+p, i]  (pair g=0)
    #   wkR[p, 1024(g-1) + 128k + i] = Wk[128k+p, 128g+i]  (pairs 1..3)
    wk0d = nc.dram_tensor("wk0", [128, KT * 128], BF16, kind="ExternalInput")
    wq0d = nc.dram_tensor("wq0", [128, KT * 128], BF16, kind="ExternalInput")
    wkRd = nc.dram_tensor("wkR", [128, 3 * KT * 128], BF16, kind="ExternalInput")
    wqRd = nc.dram_tensor("wqR", [128, 3 * KT * 128], BF16, kind="ExternalInput")
    # wv k-major: wv[p, 512k+j] = Wv[128k+p, j]
    wvd = nc.dram_tensor("wv", [128, KT * DH], BF16, kind="ExternalInput")
    # wo g-major: wo[p, 1024g+j] = Wo[128g+p, j]
    wod = nc.dram_tensor("wo", [128, 4 * D], BF16, kind="ExternalInput")
    maskt = nc.dram_tensor("maskt", [128, 128], F32, kind="ExternalInput")
    out = nc.dram_tensor("out", [S, D], F32, kind="ExternalOutput")
    # DRAM bounce rows for the reciprocal partition-broadcast (DMA cannot
    # broadcast an SBUF source, but a DRAM source row replicates fine)
    rscratch = nc.dram_tensor("rscratch", [4 * NCH * 2, CH], F32, kind="Internal")

    with tile.TileContext(nc) as tc, ExitStack() as ctx:
        const = ctx.enter_context(tc.tile_pool(name="const", bufs=1))
        xTp = ctx.enter_context(tc.tile_pool(name="xTp", bufs=1))
        wp = ctx.enter_context(tc.tile_pool(name="wp", bufs=1))
        qkp = ctx.enter_context(tc.tile_pool(name="qkp", bufs=1))
        vp = ctx.enter_context(tc.tile_pool(name="vp", bufs=1))
        ctp = ctx.enter_context(tc.tile_pool(name="ctp", bufs=1))
        pTp = ctx.enter_context(tc.tile_pool(name="pTp", bufs=4))
        rp = ctx.enter_context(tc.tile_pool(name="rp", bufs=3))
        bp = ctx.enter_context(tc.tile_pool(name="bp", bufs=3))
        osb = ctx.enter_context(tc.tile_pool(name="osb", bufs=2))
        ps_sc = ctx.enter_context(tc.tile_pool(name="ps_sc", bufs=2, space="PSUM"))
        ps_ctx = ctx.enter_context(tc.tile_pool(name="ps_ctx", bufs=2, space="PSUM"))
        ps_mm = ctx.enter_context(tc.tile_pool(name="ps_mm", bufs=2, space="PSUM"))

        # --- SBUF persistents --------------------------------------------
        xbig = xTp.tile([128, KT * S], BF16, name="xbig", tag="xbig")
        wk0 = wp.tile([128, KT * 128], BF16, name="wk0", tag="wk0")
        wq0 = wp.tile([128, KT * 128], BF16, name="wq0", tag="wq0")
        wkR = wp.tile([128, 3 * KT * 128], BF16, name="wkR", tag="wkR")
        wqR = wp.tile([128, 3 * KT * 128], BF16, name="wqR", tag="wqR")
        wv = wp.tile([128, KT * DH], BF16, name="wv", tag="wv")
        wo = wp.tile([128, 4 * D], BF16, name="wo", tag="wo")
        mask = const.tile([128, 128], F32, name="mask", tag="mask")

        def xts(k):
            return xbig[:, S * k:S * (k + 1)]

        def wk_sl(g, k):
            if g == 0:
                return wk0[:, 128 * k:128 * (k + 1)]
            return wkR[:, 1024 * (g - 1) + 128 * k:1024 * (g - 1) + 128 * (k + 1)]

        def wq_sl(g, k):
            if g == 0:
                return wq0[:, 128 * k:128 * (k + 1)]
            return wqR[:, 1024 * (g - 1) + 128 * k:1024 * (g - 1) + 128 * (k + 1)]

        # --- input DMAs, priority order ----------------------------------
        nc.sync.dma_start(wk0[:], wk0d[:])
        nc.sync.dma_start(wq0[:], wq0d[:])
        xr = xbig[:].rearrange("p (k s) -> p k s", s=S)
        nc.sync.dma_start(xr[:, :, 0:CH], xcd[0][:])
        nc.sync.dma_start(mask[:], maskt[:])
        nc.sync.dma_start(wv[:], wvd[:])
        for c in range(1, NCH):
            nc.sync.dma_start(xr[:, :, CH * c:CH * (c + 1)], xcd[c][:])
        nc.sync.dma_start(wkR[:], wkRd[:])
        nc.sync.dma_start(wqR[:], wqRd[:])
        nc.sync.dma_start(wo[:], wod[:])

        # --- persistent intermediates ------------------------------------
        qts = [qkp.tile([128, S], BF16, name=f"qt{g}", tag=f"qt{g}") for g in range(4)]
        kts = [qkp.tile([128, S], BF16, name=f"kt{g}", tag=f"kt{g}") for g in range(4)]
        vts = [vp.tile([128, 8 * 65], BF16, name=f"v{m}", tag=f"v{m}") for m in range(NKB)]
        cts = [ctp.tile([128, S], BF16, name=f"ct{g}", tag=f"ct{g}") for g in range(4)]

        # --- filler units: closures over in-flight PSUM accumulations ----
        fillers = []
        fill_pos = 0

        def pop_fill(n):
            nonlocal fill_pos
            end = min(fill_pos + n, len(fillers))
            while fill_pos < end:
                fillers[fill_pos][1]()
                fill_pos += 1

        def drain_tag(tag):
            """Run every queued unit up to and including the last one with
            this tag (correctness-of-perf: deps are real semaphores)."""
            nonlocal fill_pos
            last = -1
            for i in range(fill_pos, len(fillers)):
                if fillers[i][0] == tag:
                    last = i
            while fill_pos <= last:
                fillers[fill_pos][1]()
                fill_pos += 1

        def push_qk_units(dst, wsl, g, c):
            st8 = {}

            def a():
                ps = ps_mm.tile([128, CH], F32, name="mm", tag="mm")
                st8["ps"] = ps
                for k in range(4):
                    nc.tensor.matmul(ps[:], lhsT=wsl(g, k),
                                     rhs=xts(k)[:, CH * c:CH * (c + 1)],
                                     start=(k == 0), stop=False,
                                     skip_group_check=True)

            def b():
                ps = st8["ps"]
                for k in range(4, KT):
                    nc.tensor.matmul(ps[:], lhsT=wsl(g, k),
                                     rhs=xts(k)[:, CH * c:CH * (c + 1)],
                                     start=False, stop=(k == KT - 1),
                                     skip_group_check=True)
                nc.vector.tensor_copy(dst[:, CH * c:CH * (c + 1)], ps[:])

            tag = f"qk{g}c{c}"
            fillers.append((tag, a))
            fillers.append((tag, b))

        def push_v_units(m):
            st8 = {}

            def a():
                ps = ps_mm.tile([128, CH], F32, name="mm", tag="mm")
                st8["ps"] = ps
                for k in range(4):
                    nc.tensor.matmul(ps[:], lhsT=xts(k)[:, 128 * m:128 * (m + 1)],
                                     rhs=wv[:, DH * k:DH * (k + 1)],
                                     start=(k == 0), stop=False,
                                     skip_group_check=True)

            def b():
                ps = st8["ps"]
                for k in range(4, KT):
                    nc.tensor.matmul(ps[:], lhsT=xts(k)[:, 128 * m:128 * (m + 1)],
                                     rhs=wv[:, DH * k:DH * (k + 1)],
                                     start=False, stop=(k == KT - 1),
                                     skip_group_check=True)
                vm = vts[m]
                dst = vm[:].rearrange("p (h x) -> p h x", x=65)[:, :, 0:64]
                src = ps[:].rearrange("p (h d) -> p h d", d=64)
                nc.vector.tensor_copy(dst, src)
                ones = vm[:].rearrange("p (h x) -> p h x", x=65)[:, :, 64:65]
                nc.vector.memset(ones, 1.0)

            tag = f"v{m}"
            fillers.append((tag, a))
            fillers.append((tag, b))

        def push_outproj_units(c):
            for qb in range(4 * c, 4 * c + 4):
                st8 = {}

                def a(qb=qb, st8=st8):
                    ost = osb.tile([128, D], F32, name="ost", tag="ost")
                    ps = ps_mm.tile([128, CH], F32, name="mm", tag="mm")
                    st8["ost"], st8["ps"] = ost, ps
                    for g in range(4):
                        nc.tensor.matmul(ps[:], lhsT=cts[g][:, 128 * qb:128 * (qb + 1)],
                                         rhs=wo[:, D * g:D * g + CH],
                                         start=(g == 0), stop=(g == 3),
                                         skip_group_check=True)
                    nc.vector.tensor_copy(ost[:, 0:CH], ps[:])

                def b(qb=qb, st8=st8):
                    ost = st8["ost"]
                    ps = ps_mm.tile([128, CH], F32, name="mm", tag="mm")
                    for g in range(4):
                        nc.tensor.matmul(ps[:], lhsT=cts[g][:, 128 * qb:128 * (qb + 1)],
                                         rhs=wo[:, D * g + CH:D * (g + 1)],
                                         start=(g == 0), stop=(g == 3),
                                         skip_group_check=True)
                    nc.vector.tensor_copy(ost[:, CH:D], ps[:])
                    nc.sync.dma_start(out[128 * qb:128 * (qb + 1), :], ost[:])

                tag = f"op{c}"
                fillers.append((tag, a))
                fillers.append((tag, b))

        # --- attention ----------------------------------------------------
        def run_chunk(g, c):
            """Heads (2g, 2g+1), queries [CH*c, CH*(c+1)); software-pipelined:
            sc(j+1) and filler units are issued before pv(j)."""
            drain_tag(f"qk{g}c{c}")
            qt, ktile = qts[g], kts[g]
            nj = 4 * c + 4
            ctx_e = ps_ctx.tile([65, CH], F32, name="ctx", tag="ctx")
            ctx_o = ps_ctx.tile([65, CH], F32, name="ctx", tag="ctx")
            pts = [None] * nj

            def sc_step(j):
                d = j - 4 * c
                st = 128 * max(0, d)
                sc = ps_sc.tile([128, 2 * CH], F32, name="sc", tag="sc")
                # even head in bank 0 (cols 0:512), odd head in bank 1
                nc.tensor.matmul(
                    sc[:, st:CH], lhsT=ktile[0:64, 128 * j:128 * (j + 1)],
                    rhs=qt[0:64, CH * c + st:CH * (c + 1)])
                nc.tensor.matmul(
                    sc[:, CH + st:2 * CH], lhsT=ktile[64:128, 128 * j:128 * (j + 1)],
                    rhs=qt[64:128, CH * c + st:CH * (c + 1)])
                if d >= 0:
                    ms = slice(128 * d, 128 * (d + 1))
                    nc.vector.tensor_tensor(sc[:, ms], sc[:, ms], mask[:],
                                            op=mybir.AluOpType.add)
                    ms2 = slice(CH + 128 * d, CH + 128 * (d + 1))
                    nc.vector.tensor_tensor(sc[:, ms2], sc[:, ms2], mask[:],
                                            op=mybir.AluOpType.add)
                pt = pTp.tile([128, 2 * CH], BF16, name="pT", tag="pT")
                # one exp covers both heads (cols CH:CH+st are unread junk)
                nc.scalar.activation(pt[:, st:], sc[:, st:],
                                     mybir.ActivationFunctionType.Exp, scale=0.125)
                pts[j] = (pt, st)

            def pv_step(j):
                pt, st = pts[j]
                he, ho = 2 * g, 2 * g + 1
                nc.tensor.matmul(ctx_e[:, st:], lhsT=vts[j][:, 65 * he:65 * he + 65],
                                 rhs=pt[:, st:CH], start=(j == 0), stop=(j == nj - 1),
                                 skip_group_check=True)
                nc.tensor.matmul(ctx_o[:, st:], lhsT=vts[j][:, 65 * ho:65 * ho + 65],
                                 rhs=pt[:, CH + st:2 * CH], start=(j == 0),
                                 stop=(j == nj - 1), skip_group_check=True)

            sc_step(0)
            for j in range(nj):
                if j + 1 < nj:
                    drain_tag(f"v{j + 1}")  # usually a no-op; keeps pv fed
                    sc_step(j + 1)
                pop_fill(2)
                pv_step(j)

            # normalize by the fused denominator row and store into ctxT (bf16)
            for par, (ctx_ps, rows) in enumerate(
                    ((ctx_e, slice(0, 64)), (ctx_o, slice(64, 128)))):
                idx = (g * NCH + c) * 2 + par
                stg = rp.tile([65, CH], F32, name="stg", tag="stg")
                nc.vector.tensor_copy(stg[:], ctx_ps[:])
                # custom-DVE input must sit at partition 0: copy the sums row
                srow = rp.tile([1, CH], F32, name="srow", tag="srow")
                nc.vector.tensor_copy(srow[:], stg[64:65, :])
                rc = rp.tile([1, CH], F32, name="recip", tag="recip")
                nc.vector.reciprocal_approx_fast(rc[:], srow[:])
                nc.sync.dma_start(rscratch[idx:idx + 1, :], rc[:])
                bc = bp.tile([64, CH], F32, name="bcast", tag="bcast")
                nc.sync.dma_start(bc[:], rscratch[idx:idx + 1, :].broadcast_to([64, CH]))
                nc.vector.tensor_tensor(cts[g][rows, CH * c:CH * (c + 1)],
                                        stg[0:64, :], bc[:],
                                        op=mybir.AluOpType.mult)

        # --- schedule -----------------------------------------------------
        # pair-0 K/Q chunk 0 directly (first PE work, ~8us in)
        push_qk_units(kts[0], wk_sl, 0, 0)
        push_qk_units(qts[0], wq_sl, 0, 0)
        drain_tag("qk0c0")
        # filler queue for the g=0 phase: V blocks as needed + later chunks
        for m in range(4):
            push_v_units(m)
        for c in range(1, NCH):
            push_qk_units(kts[0], wk_sl, 0, c)
            push_qk_units(qts[0], wq_sl, 0, c)
            for m in range(4 * c, 4 * c + 4):
                push_v_units(m)
        # pair-1 projections also as g=0-phase filler
        for c in range(NCH):
            push_qk_units(kts[1], wk_sl, 1, c)
            push_qk_units(qts[1], wq_sl, 1, c)

        for g in range(4):
            if g >= 1:
                # next pair's projections pop as filler during this phase
                for c in range(NCH):
                    if g + 1 < 4:
                        push_qk_units(kts[g + 1], wk_sl, g + 1, c)
                        push_qk_units(qts[g + 1], wq_sl, g + 1, c)
            for c in range(NCH):
                run_chunk(g, c)
                if g == 3:
                    push_outproj_units(c)
        pop_fill(len(fillers))  # flush remaining out-proj units

    nc.compile()
    return nc


def _get_module():
    global _CACHED
    if _CACHED is None:
        _CACHED = build_module()
    return _CACHED


def _causal_mask_tile():
    k = np.arange(128)[:, None]
    q = np.arange(128)[None, :]
    return np.where(k <= q, 0.0, NEG).astype(np.float32)


def make_in_maps(inputs, Wq, Wk, Wv, Wo):
    """Host-side packing into the dram-tensor layouts (see build_module)."""
    bf = ml_dtypes.bfloat16
    mask = _causal_mask_tile()
    in_maps = []
    for core in range(NCORES):
        b, hh = core // 2, core % 2
        cols = slice(DH * hh, DH * (hh + 1))
        wqh = Wq[:, cols].reshape(KT, 128, 4, 128)   # [k,p,g,i]
        wkh = Wk[:, cols].reshape(KT, 128, 4, 128)
        wvh = Wv[:, cols].reshape(KT, 128, DH)       # [k,p,j]
        woh = Wo[cols, :].reshape(4, 128, D)         # [g,p,j]
        xT = inputs[b].T.reshape(KT, 128, NCH, CH)   # [k,p,c,s]
        m = {
            "wk0": wkh[:, :, 0, :].transpose(1, 0, 2).reshape(128, KT * 128),
            "wq0": wqh[:, :, 0, :].transpose(1, 0, 2).reshape(128, KT * 128),
            "wkR": wkh[:, :, 1:, :].transpose(1, 2, 0, 3).reshape(128, 3 * KT * 128),
            "wqR": wqh[:, :, 1:, :].transpose(1, 2, 0, 3).reshape(128, 3 * KT * 128),
            "wv": wvh.transpose(1, 0, 2).reshape(128, KT * DH),
            "wo": woh.transpose(1, 0, 2).reshape(128, 4 * D),
        }
        for c in range(NCH):
            m[f"xc{c}"] = xT[:, :, c, :].transpose(1, 0, 2).reshape(128, KT * CH)
        m = {k: np.ascontiguousarray(v).astype(bf) for k, v in m.items()}
        m["maskt"] = mask
        in_maps.append(m)
    return in_maps


def kernel(inputs, Wq, Wk, Wv, Wo, bo):
    inputs = np.asarray(inputs, dtype=np.float32)
    Wq = np.asarray(Wq, dtype=np.float32)
    Wk = np.asarray(Wk, dtype=np.float32)
    Wv = np.asarray(Wv, dtype=np.float32)
    Wo = np.asarray(Wo, dtype=np.float32)
    bo = np.asarray(bo, dtype=np.float32)

    in_maps = make_in_maps(inputs, Wq, Wk, Wv, Wo)
    nc = _get_module()
    res = bass_utils.run_bass_kernel_spmd(nc, in_maps, core_ids=list(range(NCORES)))
    outs = [r["out"] for r in res.results]

    full = np.empty((B, S, D), dtype=np.float32)
    for b in range(B):
        full[b] = outs[2 * b] + outs[2 * b + 1] + bo[None, :]
    return full
